# revision 9
# baseline (speedup 1.0000x reference)
"""Trainium2 Bass kernel for the CSD loss function (TensorE MAC version).

Reference math (N = 4194304 samples, C = 10 classes):
    counts[c]  = sum_i [target_i == c]
    nom[i]     = outputs[i] . counts
    result     = log(||outputs||_F * sqrt(N)) - mean(log(nom))

Per-core layout (8-way data parallel, NS = 524288 rows/core):
  The host casts x to fp8e4m3 (halves the dominant HBM stream; W stays
  bf16 so the exact f32 counts only lose bf16 rounding) and permutes it
  so that every 128 consecutive flat elements of a 128-row group land on
  the 128 SBUF partitions ("pre-transposed" blocks).  With that layout
  the per-row dot product with the count vector becomes a chain of
  128x128 x 128x512 TensorE matmuls whose bf16 stationary W_k holds the
  counts scattered in a compile-time pattern: W_k[q, r'] =
  counts[(128k+q)%10] iff (128k+q)//10 == r'.  Ten matmuls accumulate
  one PSUM bank [128, 512] holding 65536 nominators; ScalarE Ln+accum
  reduces it.  Measured rel err vs the f32 reference: 2.6e-4.

  counts: 9 DVE is_equal compares on the bf16 target shard (4x mode,
  class 9 by subtraction from the per-partition total),
  partition-reduced by a tiny TensorE matmul against ones, AllReduced
  across the 8 cores as [10, 1] f32, then expanded to per-partition
  cyclic count columns by 5 tiny matmuls against host-shipped selection
  matrices.  W_k = mask_k * counts_q (DVE tensor_scalar, 4x mode).

  squares: split between ScalarE (Square+accum) and DVE
  (scalar_tensor_tensor x*x with accum; tensor_tensor_reduce crashes
  this device), tuned so both engines stay under the DMA roofline.
  PE warmup matmuls keep the tensor engine clocked up before the real
  matmuls start.
"""

import numpy as np
import ml_dtypes

import concourse.bass as bass
import concourse.tile as tile
from concourse import bacc, mybir
from concourse.bass_utils import run_bass_kernel_spmd

F32 = mybir.dt.float32
BF16 = mybir.dt.bfloat16
FP8 = mybir.dt.float8e4
ALU = mybir.AluOpType
ACTFN = mybir.ActivationFunctionType

NCORES = 8
N = 4194304
C = 10
NS = N // NCORES          # rows per core = 524288
P = 128                   # SBUF partitions
RP = NS // P              # target elems per partition = 4096
XF = NS * C // P          # x free size per partition = 40960

NSG = 8                   # supergroups per core
SGF = XF // NSG           # free cols per supergroup = 5120 (10 k-blocks x 512)
MMF = 512                 # moving cols per matmul (1 PSUM bank of f32)
NK = 10                   # k-blocks (matmuls) per supergroup

# fraction of square-sum columns handled by ScalarE (rest on DVE TTR)
SQ_ACT_COLS = 3328        # of SGF=5120; DVE TTR takes the remaining 1792
N_WARM = 56               # PE warmup matmuls (keep clock ramped)
EARLY_SQ = 2              # supergroups whose DVE square runs before W build

X_FP8 = True            # x shipped as fp8e4m3 (W/masks stay bf16)
TRACE = False
LAST_RESULT = None


def _masks_sel_host():
    """Host-side constants: mask_k scatter patterns and SEL phase matrices."""
    masks = np.zeros((P, NK * P), dtype=ml_dtypes.bfloat16)  # [q, k*128+r']
    for k in range(NK):
        for q in range(P):
            u = 128 * k + q
            masks[q, k * P + u // C] = 1.0
    sel = np.zeros((C, 5 * P), dtype=np.float32)             # [c, j*128+q]
    for j in range(5):
        for q in range(P):
            sel[(128 * j + q) % C, j * P + q] = 1.0
    ones = np.ones((P, 1), dtype=np.float32)
    return masks, sel, ones


def _prep_x(outputs):
    """Cast to bf16 and permute to the transposed-block layout.

    Element (core n, partition q, free sg*5120 + k*512 + f) =
    x[n*NS + sg*65536 + f*128 + (128k+q)//10, (128k+q)%10].
    """
    xdt = ml_dtypes.float8_e4m3 if X_FP8 else ml_dtypes.bfloat16
    xb = outputs.astype(xdt)
    v = xb.reshape(NCORES, NSG, MMF, P, C)        # [n, sg, f, rr, c]
    v = np.ascontiguousarray(v.transpose(0, 1, 3, 4, 2))  # [n, sg, rr, c, f]
    v = v.reshape(NCORES, NSG, NK, P, MMF)        # [n, sg, k, q, f]
    v = np.ascontiguousarray(v.transpose(0, 3, 1, 2, 4))  # [n, q, sg, k, f]
    return v.reshape(NCORES, P, XF)


def build(collective=True, repeat=1, xbufs=None):
    if xbufs is None:
        xbufs = NSG if repeat == 1 else 4
    nc = bacc.Bacc("TRN2", target_bir_lowering=False, debug=False,
                   num_devices=NCORES)
    XDT = FP8 if X_FP8 else BF16
    x = nc.dram_tensor("x", [P, XF], XDT, kind="ExternalInput")
    t = nc.dram_tensor("t", [P, RP], BF16, kind="ExternalInput")
    mk = nc.dram_tensor("mk", [P, NK * P], BF16, kind="ExternalInput")
    sl = nc.dram_tensor("sl", [C, 5 * P], F32, kind="ExternalInput")
    on = nc.dram_tensor("on", [P, 1], F32, kind="ExternalInput")
    # per rep: cols [0:8]=ln, [8:16]=sq_act, [16:24]=sq_dve
    part_out = nc.dram_tensor("part", [P, 24 * repeat], F32,
                              kind="ExternalOutput")
    cnt_out = nc.dram_tensor("cnt", [C, 1], F32, kind="ExternalOutput")

    with tile.TileContext(nc) as tc:
        with (
            tc.tile_pool(name="const", bufs=1) as constp,
            tc.tile_pool(name="tgt", bufs=1) as tgtp,
            tc.tile_pool(name="cnt", bufs=1) as cntp,
            tc.tile_pool(name="w", bufs=1) as wp,
            tc.tile_pool(name="xp", bufs=xbufs) as xp,
            tc.tile_pool(name="scr", bufs=1) as scrp,
            tc.tile_pool(name="res", bufs=1) as resp,
            tc.tile_pool(name="pwarm", bufs=1, space="PSUM") as pwarm,
            tc.tile_pool(name="pcnt", bufs=1, space="PSUM") as pcnt,
            tc.tile_pool(name="pcq", bufs=1, space="PSUM") as pcq,
            tc.tile_pool(name="pmm", bufs=3, space="PSUM") as pmm,
            tc.tile_pool(name="dram", bufs=2, space="DRAM") as dram,
        ):
            # ---- DMAs: tiny consts first, then target, then x stream ----
            masks = constp.tile([P, NK * P], BF16, tag="mk")
            nc.sync.dma_start(masks[:], mk[:])
            sel = constp.tile([C, 5 * P], F32, tag="sl")
            nc.sync.dma_start(sel[:], sl[:])
            ones = constp.tile([P, 1], F32, tag="on")
            nc.sync.dma_start(ones[:], on[:])
            tgt = tgtp.tile([P, RP], BF16)
            nc.sync.dma_start(tgt[:], t[:])

            xt0 = []          # rep-0 supergroup tiles
            for sg in range(NSG):
                xtile = xp.tile([P, SGF], XDT, tag="x")
                nc.sync.dma_start(xtile[:], x[:, sg * SGF:(sg + 1) * SGF])
                xt0.append(xtile)

            # ---- PE warmup: keep the tensor engine busy/ramped until the
            # count-reduce matmul is ready (junk results into a scratch bank).
            warm = pwarm.tile([P, MMF], F32)
            for _ in range(N_WARM):
                nc.tensor.matmul(warm[:], masks[:, 0:P], masks[:, 0:MMF])

            # ---- class counts on DVE (4x compare mode) ----
            # only 9 compares: each partition holds exactly RP targets, so
            # counts[9] = RP - sum(counts[0:9]).
            ind = tgtp.tile([P, RP], BF16, tag="ind")
            cnt_cols = cntp.tile([P, C], F32)
            for c in range(C - 1):
                nc.vector.tensor_scalar(
                    ind[:], tgt[:], float(c), None, ALU.is_equal, ALU.add,
                    accum_out=cnt_cols[:, c:c + 1])
            csum = cntp.tile([P, 1], F32, tag="csum")
            nc.vector.tensor_reduce(csum[:], cnt_cols[:, 0:C - 1],
                                    mybir.AxisListType.X, ALU.add)
            nc.vector.tensor_scalar(cnt_cols[:, C - 1:C], csum[:], -1.0,
                                    float(RP), ALU.mult, ALU.add)

            # partition-reduce via matmul: [128,10]^T @ ones -> [10, 1]
            cntP = pcnt.tile([C, 1], F32)
            nc.tensor.matmul(cntP[:], cnt_cols[:], ones[:])
            cnt_loc = cntp.tile([C, 1], F32, tag="cloc")
            nc.vector.tensor_copy(cnt_loc[:], cntP[:])

            # AllReduce the [10, 1] count vector across the 8 cores
            cc_in = dram.tile([C, 1], F32, tag="ccin")
            cc_ot = dram.tile([C, 1], F32, tag="ccout")
            nc.gpsimd.dma_start(cc_in[:], cnt_loc[:])
            if collective:
                nc.gpsimd.collective_compute(
                    "AllReduce", ALU.add,
                    replica_groups=[list(range(NCORES))],
                    ins=[cc_in.opt()], outs=[cc_ot.opt()])
            else:
                nc.gpsimd.dma_start(cc_ot[:], cc_in[:])
            cnt_glob = cntp.tile([C, 1], F32, tag="cglob")
            nc.gpsimd.dma_start(cnt_glob[:], cc_ot[:])

            # ---- partials + scratch ----
            parts = resp.tile([P, 24 * repeat], F32)
            scr = scrp.tile([P, SGF - SQ_ACT_COLS], BF16)
            sq_dummy = constp.tile([P, 1], F32, tag="sqdummy")
            ln_dummy = constp.tile([P, 1], F32, tag="lndummy")

            def emit_sq_act(xtile, sg, rep):
                nc.scalar.activation(
                    sq_dummy.broadcast_to((P, SQ_ACT_COLS)),
                    xtile[:, 0:SQ_ACT_COLS], ACTFN.Square,
                    accum_out=parts[:, 24 * rep + 8 + sg:24 * rep + 9 + sg])

            def emit_sq_dve(xtile, sg, rep):
                # out = (x * 1.0) * x, accum_out = sum(out) — same DVE
                # instruction family as the count compares (HW-safe).
                nc.vector.scalar_tensor_tensor(
                    scr[:], xtile[:, SQ_ACT_COLS:SGF], 1.0,
                    xtile[:, SQ_ACT_COLS:SGF],
                    ALU.mult, ALU.mult,
                    accum_out=parts[:, 24 * rep + 16 + sg:24 * rep + 17 + sg])

            # early DVE squares soak up the collective wait
            for sg in range(EARLY_SQ):
                emit_sq_dve(xt0[sg], sg, 0)
                emit_sq_act(xt0[sg], sg, 0)

            # ---- counts_q: 5 tiny matmuls SEL_j^T @ counts -> [128, 1] ----
            cqP = pcq.tile([P, 8], F32)
            for j in range(5):
                nc.tensor.matmul(cqP[:, j:j + 1], sel[:, j * P:(j + 1) * P],
                                 cnt_glob[:])

            # ---- W_k = mask_k * counts_q[phase(k)]  (DVE 4x) ----
            W = wp.tile([P, NK * P], BF16)
            for k in range(NK):
                ph = k % 5
                nc.vector.tensor_scalar(
                    W[:, k * P:(k + 1) * P], masks[:, k * P:(k + 1) * P],
                    cqP[:, ph:ph + 1], None, ALU.mult)

            # ---- main stream: 10 matmuls + Ln per supergroup ----
            for rep in range(repeat):
                for sg in range(NSG):
                    if rep == 0:
                        xtile = xt0[sg]
                    else:
                        xtile = xp.tile([P, SGF], XDT, tag="x")
                        nc.sync.dma_start(
                            xtile[:], x[:, sg * SGF:(sg + 1) * SGF])
                    ps = pmm.tile([P, MMF], F32, tag="mm")
                    for k in range(NK):
                        nc.tensor.matmul(
                            ps[:], W[:, k * P:(k + 1) * P],
                            xtile[:, k * MMF:(k + 1) * MMF],
                            start=(k == 0), stop=(k == NK - 1))
                    nc.scalar.activation(
                        ln_dummy.broadcast_to((P, MMF)), ps[:], ACTFN.Ln,
                        accum_out=parts[:, 24 * rep + sg:24 * rep + sg + 1])
                    if rep > 0 or sg >= EARLY_SQ:
                        emit_sq_act(xtile, sg, rep)
                        emit_sq_dve(xtile, sg, rep)

            nc.sync.dma_start(part_out[:], parts[:])
            nc.sync.dma_start(cnt_out[:], cnt_glob[:])

    nc.compile()
    return nc


_NC = None


def _get_nc():
    global _NC
    if _NC is None:
        _NC = build()
    return _NC


def make_in_maps(outputs, target):
    xs = _prep_x(outputs)
    ts = target.astype(ml_dtypes.bfloat16).reshape(NCORES, P, RP)
    masks, sel, ones = _masks_sel_host()
    return [{"x": xs[c], "t": ts[c], "mk": masks, "sl": sel, "on": ones}
            for c in range(NCORES)]


def reduce_outputs(results, repeat=1):
    ln_total = 0.0
    sq_total = 0.0
    for r in results:
        pr = np.asarray(r["part"], dtype=np.float64).reshape(P, repeat, 24)
        ln_total += float(pr[:, :, 0:8].sum())
        sq_total += float(pr[:, :, 8:24].sum())
    ln_total /= repeat
    sq_total /= repeat
    return np.float32(
        np.log(np.sqrt(sq_total) * np.sqrt(float(N))) - ln_total / N)


def kernel(outputs, target):
    global LAST_RESULT
    outputs = np.ascontiguousarray(np.asarray(outputs, dtype=np.float32))
    target = np.asarray(target)
    assert outputs.shape == (N, C) and target.shape == (N,)

    in_maps = make_in_maps(outputs, target)
    res = run_bass_kernel_spmd(
        _get_nc(), in_maps, core_ids=list(range(NCORES)), trace=TRACE)
    LAST_RESULT = res
    return reduce_outputs(res.results)
